# revision 1
# baseline (speedup 1.0000x reference)
"""DeepSets segment-reduce kernel for 8 Trainium2 NeuronCores.

Math:  y = segment_sum(tanh(x @ W1.T + b1), batch) @ W2.T + b2

Strategy (all 8 cores run the SAME program, SPMD; per-core data differs):
  - Host pads every segment to a multiple of B=16 nodes (zero rows), groups
    128 consecutive segments into a "window" (4 windows/core x 8 cores),
    pads every window to a uniform node count, and pre-transposes x so the
    device sees xT [128(h), Nc] per core - no on-device transposes.
  - Device, per 1024-node tile:
      PE:  phiT_pre = W1T_chunk.T @ xT_tile  (2 o-chunks x 2 q-halves, f32)
      ACT: phiT = tanh(psum + b1_chunk)  PSUM->SBUF, bf16 out (fused bias)
      DVE: 3 rounds of pairwise adds (bf16 2x mode) reduce 16-node blocks
           to 8-node half-block sums L05 [128(o-chunk), 128 cols]
      PE:  zT = L05_c0.T @ W2T_c0 + L05_c1.T @ W2T_c1   (fc2 applied to
           half-block sums - fc2 commutes with segment-sum by linearity)
      DVE: copy zT PSUM -> SBUF (bf16)
      PE:  y_win += S_tile.T @ zT   (S = host-built one-hot mapping
           half-block-cols -> segment-cols; accumulates in PSUM per window)
  - Host: y = concat(core outputs) + b2 - npad[g] * (tanh(b1) @ W2.T)
    (exact linear correction for the zero-pad rows, whose phi is tanh(b1)).
"""

import os
import sys

for _p in ("/opt/trn_rl_repo", "/root/.axon_site/_ro/trn_rl_repo"):
    if os.path.isdir(_p) and _p not in sys.path:
        sys.path.append(_p)

import numpy as np
import ml_dtypes

G = 4096          # segments
H = 128           # input feature dim
O = 256           # hidden dim (2*H)
B = 16            # tree block size (nodes)
PADB = 16         # segment padding granularity: the DVE tree's pairwise
                  # halving yields even/odd interleaved half-sums within each
                  # 16-block, so segment boundaries must be 16-aligned
HB = 8            # half-block: one L05 column sums HB nodes
T = 3072          # main-loop big tile, in nodes
SUB = 1536        # PSUM/ACT subtile, in nodes
LT = 1024         # ltile (combine granularity), in nodes
SEGS_PER_WIN = 128
N_CORES = 8
WINS_PER_CORE = 4
N_WINS = N_CORES * WINS_PER_CORE  # 32

_BF16 = ml_dtypes.bfloat16


def _prep_host(x, batch):
    """Pad/shard/transpose inputs. Returns per-core arrays + metadata.

    Segments are bin-packed (LPT greedy) into 32 windows of exactly 128
    segments each to minimize the max window node count; window w lives on
    core w//4.  seg_order[w*128+j] = original segment id of device row
    (w, j); host assembly un-permutes.
    """
    x = np.asarray(x, dtype=np.float32)
    batch = np.asarray(batch, dtype=np.int64)
    N = x.shape[0]

    cnt = np.bincount(batch, minlength=G).astype(np.int64)     # [G]
    plen = ((cnt + PADB - 1) // PADB) * PADB                   # [G]

    # --- LPT bin-pack segments into N_WINS windows of SEGS_PER_WIN each
    order = np.argsort(-plen, kind="stable")
    loads = np.zeros(N_WINS, dtype=np.int64)
    fill = np.zeros(N_WINS, dtype=np.int64)
    win_of_seg = np.empty(G, dtype=np.int64)
    col_of_seg = np.empty(G, dtype=np.int64)
    open_bins = list(range(N_WINS))
    import heapq
    heap = [(0, w) for w in range(N_WINS)]
    heapq.heapify(heap)
    for g in order:
        while True:
            load, w = heapq.heappop(heap)
            if fill[w] < SEGS_PER_WIN:
                break
        win_of_seg[g] = w
        col_of_seg[g] = fill[w]
        fill[w] += 1
        loads[w] = load + plen[g]
        if fill[w] < SEGS_PER_WIN:
            heapq.heappush(heap, (loads[w], w))
    Lw = int(((loads.max() + LT - 1) // LT) * LT)              # nodes/window
    Nc = (WINS_PER_CORE * Lw)                                  # nodes/core

    # start offset of each segment inside its window: order segments within
    # a window by column index, accumulate plen
    seg_start_in_win = np.zeros(G, dtype=np.int64)
    for w in range(N_WINS):
        segs = np.where(win_of_seg == w)[0]
        segs = segs[np.argsort(col_of_seg[segs])]
        seg_start_in_win[segs] = np.concatenate(
            ([0], np.cumsum(plen[segs])[:-1]))

    # destination position of each node (rank within its segment)
    seg_first = np.concatenate(([0], np.cumsum(cnt)[:-1]))     # first node idx
    if np.all(np.diff(batch) >= 0):
        idx_in_seg = np.arange(N) - seg_first[batch]
    else:  # defensive: reference always sorts, but handle unsorted too
        sort_idx = np.argsort(batch, kind="stable")
        idx_in_seg = np.empty(N, dtype=np.int64)
        idx_in_seg[sort_idx] = np.arange(N) - seg_first[batch[sort_idx]]
    wn = win_of_seg[batch]
    core_of_node = wn // WINS_PER_CORE
    pos = (wn % WINS_PER_CORE) * Lw + seg_start_in_win[batch] + idx_in_seg

    # scatter: xT[core, :, pos] = x[n]  (bf16 for full-rate PE + half DMA)
    flat = core_of_node * Nc + pos
    xpad = np.zeros((N_CORES * Nc, H), dtype=_BF16)
    xpad[flat] = x.astype(_BF16)
    xT = np.ascontiguousarray(xpad.reshape(N_CORES, Nc, H).transpose(0, 2, 1))

    # S matrices: per core, per ltile (=128 l05 cols =1024 nodes):
    # S[lrow, segcol] = 1 if the l05 col belongs to that segment's column
    L = Nc // HB                      # l05 cols per core
    nlt = Nc // (HB * SEGS_PER_WIN)   # ltiles per core
    ntiles = Nc // T                  # main tiles per core
    seg_of_col = np.full((N_CORES, L), -1, dtype=np.int64)
    col_start = ((win_of_seg % WINS_PER_CORE) * (Lw // HB)
                 + seg_start_in_win // HB)
    ncols_seg = plen // HB
    core_of_seg = win_of_seg // WINS_PER_CORE
    for g in range(G):
        if ncols_seg[g] > 0:
            c = core_of_seg[g]
            s = col_start[g]
            seg_of_col[c, s:s + ncols_seg[g]] = col_of_seg[g]
    S = np.zeros((N_CORES, nlt, SEGS_PER_WIN, SEGS_PER_WIN), dtype=np.float32)
    lt_of_col = np.arange(L) // SEGS_PER_WIN
    row_of_col = np.arange(L) % SEGS_PER_WIN
    for c in range(N_CORES):
        mask = seg_of_col[c] >= 0
        S[c, lt_of_col[mask], row_of_col[mask], seg_of_col[c, mask]] = 1.0
    S = S.astype(_BF16)

    npad = (plen - cnt).astype(np.float32)                     # [G]
    # device row (w*128+j) -> original segment id
    seg_order = np.empty(G, dtype=np.int64)
    seg_order[win_of_seg * SEGS_PER_WIN + col_of_seg] = np.arange(G)
    return xT, S, Nc, ntiles, npad, seg_order


def _build_program(Nc, ntiles):
    """Build + compile the (uniform, SPMD) Bass/Tile program for one core.

    Big tiles of T=3072 nodes (ragged tail allowed: 1024/2048); each big
    tile splits into 1536-node ACT subtiles (1024 in ragged tails) and
    1024-node ltiles for fc2/combine.
    """
    from contextlib import ExitStack
    import concourse.tile as tile
    from concourse import bacc, mybir

    f32 = mybir.dt.float32
    bf16 = mybir.dt.bfloat16
    nlt = Nc // LT                    # ltiles per core
    lt_per_win = nlt // WINS_PER_CORE

    # big tile layout: sizes + subtile splits
    sizes = []
    off = 0
    while off < Nc:
        ts = min(T, Nc - off)
        sizes.append(ts)
        off += ts
    if sizes[-1] == T:  # short final tile => shorter serial tail after last ACT
        sizes[-1] = T - LT
        sizes.append(LT)
    ntiles = len(sizes)

    def subsplit(ts):
        if ts % SUB == 0:
            return [SUB] * (ts // SUB)
        assert ts % LT == 0
        return [LT] * (ts // LT)

    nc = bacc.Bacc("TRN2", target_bir_lowering=False, debug=False)
    x_d = nc.dram_tensor("xt", [H, Nc], bf16, kind="ExternalInput").ap()
    w1t_d = nc.dram_tensor("w1t", [H, O], bf16, kind="ExternalInput").ap()
    w2t_d = nc.dram_tensor("w2t", [2, H, H], bf16, kind="ExternalInput").ap()
    b1_d = nc.dram_tensor("b1c", [2, H, 1], f32, kind="ExternalInput").ap()
    s_d = nc.dram_tensor("smat", [nlt, SEGS_PER_WIN, SEGS_PER_WIN], bf16,
                         kind="ExternalInput").ap()
    y_d = nc.dram_tensor("y", [WINS_PER_CORE * SEGS_PER_WIN, H], f32,
                         kind="ExternalOutput").ap()

    with tile.TileContext(nc) as tc:
        with ExitStack() as ctx:
            singles = ctx.enter_context(tc.tile_pool(name="singles", bufs=1))
            xpool = ctx.enter_context(tc.tile_pool(name="xpool", bufs=4))
            phipool = ctx.enter_context(tc.tile_pool(name="phipool", bufs=3))
            treepool = ctx.enter_context(tc.tile_pool(name="treepool", bufs=2))
            l05pool = ctx.enter_context(tc.tile_pool(name="l05pool", bufs=2))
            spool = ctx.enter_context(tc.tile_pool(name="spool", bufs=4))
            zpool = ctx.enter_context(tc.tile_pool(name="zpool", bufs=4))
            ypool = ctx.enter_context(tc.tile_pool(name="ypool", bufs=2))
            pspool = ctx.enter_context(
                tc.tile_pool(name="pspool", bufs=1, space="PSUM"))
            zps_pool = ctx.enter_context(
                tc.tile_pool(name="zps", bufs=1, space="PSUM"))
            yps_pool = ctx.enter_context(
                tc.tile_pool(name="yps", bufs=1, space="PSUM"))

            w1t = singles.tile([H, O], bf16)
            nc.sync.dma_start(out=w1t[:], in_=w1t_d[:])
            w2t0 = singles.tile([H, H], bf16)
            nc.sync.dma_start(out=w2t0[:], in_=w2t_d[0])
            w2t1 = singles.tile([H, H], bf16)
            nc.sync.dma_start(out=w2t1[:], in_=w2t_d[1])
            b1c0 = singles.tile([H, 1], f32)
            nc.sync.dma_start(out=b1c0[:], in_=b1_d[0])
            b1c1 = singles.tile([H, 1], f32)
            nc.sync.dma_start(out=b1c1[:], in_=b1_d[1])

            zps2 = zps_pool.tile([SEGS_PER_WIN, 2, H], f32)
            yps2 = yps_pool.tile([SEGS_PER_WIN, 2, H], f32)
            node0 = 0
            lt = 0                     # global ltile counter
            for t, ts in enumerate(sizes):
                # ---- load xT big tile
                xt = xpool.tile([H, T], bf16, tag="xt")
                nc.sync.dma_start(out=xt[:, 0:ts],
                                  in_=x_d[:, node0:node0 + ts])

                # ---- fc1 (bf16) + tanh per subtile
                phi0 = phipool.tile([H, T], bf16, tag="phi0")
                phi1 = phipool.tile([H, T], bf16, tag="phi1")
                qoff = 0
                for ss in subsplit(ts):
                    psA = pspool.tile([H, SUB], f32, tag="psA")
                    psB = pspool.tile([H, SUB], f32, tag="psB")
                    for hh in range(ss // 512):
                        sl = slice(qoff + hh * 512, qoff + (hh + 1) * 512)
                        osl = slice(hh * 512, (hh + 1) * 512)
                        nc.tensor.matmul(psA[:, osl], lhsT=w1t[:, 0:H],
                                         rhs=xt[:, sl], start=True, stop=True)
                    for hh in range(ss // 512):
                        sl = slice(qoff + hh * 512, qoff + (hh + 1) * 512)
                        osl = slice(hh * 512, (hh + 1) * 512)
                        nc.tensor.matmul(psB[:, osl], lhsT=w1t[:, H:O],
                                         rhs=xt[:, sl], start=True, stop=True)
                    qsl = slice(qoff, qoff + ss)
                    nc.scalar.activation(phi0[:, qsl], psA[:, 0:ss],
                                         mybir.ActivationFunctionType.Tanh,
                                         bias=b1c0[:], scale=1.0)
                    nc.scalar.activation(phi1[:, qsl], psB[:, 0:ss],
                                         mybir.ActivationFunctionType.Tanh,
                                         bias=b1c1[:], scale=1.0)
                    qoff += ss

                # ---- DVE tree over the big tile: 16 -> 8 -> 4 -> 2
                nb = ts // B
                l05s = []
                for ci, phi in enumerate((phi0, phi1)):
                    p3 = phi[:, 0:ts].rearrange("p (nb w) -> p nb w", w=B)
                    s1 = treepool.tile([H, T // B, 8], bf16, tag=f"s1c{ci}")
                    nc.vector.tensor_add(s1[:, 0:nb, :], p3[:, :, 0:8],
                                         p3[:, :, 8:16])
                    s2 = treepool.tile([H, T // B, 4], bf16, tag=f"s2c{ci}")
                    nc.vector.tensor_add(s2[:, 0:nb, :], s1[:, 0:nb, 0:4],
                                         s1[:, 0:nb, 4:8])
                    l05 = l05pool.tile([H, T // B, 2], bf16, tag=f"l05c{ci}")
                    nc.vector.tensor_add(l05[:, 0:nb, :], s2[:, 0:nb, 0:2],
                                         s2[:, 0:nb, 2:4])
                    l05s.append(l05)

                # ---- fc2 + combine per ltile (128 l05 cols = 1024 nodes)
                NBL = LT // B          # blocks per ltile (64)
                for q in range(ts // LT):
                    w = lt // lt_per_win
                    bsl = slice(q * NBL, (q + 1) * NBL)
                    zps = zps2[:, lt % 2, :]
                    nc.tensor.matmul(
                        zps,
                        lhsT=l05s[0][:, bsl, :].rearrange("p a b -> p (a b)"),
                        rhs=w2t0[:], start=True, stop=False)
                    nc.tensor.matmul(
                        zps,
                        lhsT=l05s[1][:, bsl, :].rearrange("p a b -> p (a b)"),
                        rhs=w2t1[:], start=False, stop=True)
                    zsb = zpool.tile([SEGS_PER_WIN, H], bf16)
                    nc.vector.tensor_copy(zsb[:], zps)

                    st = spool.tile([SEGS_PER_WIN, SEGS_PER_WIN], bf16)
                    nc.sync.dma_start(out=st[:], in_=s_d[lt])
                    w_cur = lt // lt_per_win
                    yps = yps2[:, w_cur % 2, :]
                    nc.tensor.matmul(yps, lhsT=st[:], rhs=zsb[:],
                                     start=(lt % lt_per_win == 0),
                                     stop=(lt % lt_per_win == lt_per_win - 1))
                    if lt % lt_per_win == lt_per_win - 1:
                        ysb = ypool.tile([SEGS_PER_WIN, H], f32)
                        nc.vector.tensor_copy(ysb[:], yps)
                        nc.sync.dma_start(
                            out=y_d[w_cur * SEGS_PER_WIN:
                                    (w_cur + 1) * SEGS_PER_WIN, :],
                            in_=ysb[:])
                    lt += 1
                node0 += ts

    nc.compile()
    return nc


class _Runner:
    """Persistent jitted SPMD executor over jax.devices()[:8]."""

    def __init__(self, nc):
        import jax
        from jax.sharding import Mesh, PartitionSpec
        from jax.experimental.shard_map import shard_map
        from concourse import mybir
        from concourse.bass2jax import (_bass_exec_p, install_neuronx_cc_hook,
                                        partition_id_tensor)
        install_neuronx_cc_hook()
        self.jax = jax
        self.nc = nc
        in_names, out_names, out_avals, zero_outs = [], [], [], []
        partition_name = (nc.partition_id_tensor.name
                          if nc.partition_id_tensor else None)
        for alloc in nc.m.functions[0].allocations:
            if not isinstance(alloc, mybir.MemoryLocationSet):
                continue
            name = alloc.memorylocations[0].name
            if alloc.kind == "ExternalInput":
                if name != partition_name:
                    in_names.append(name)
            elif alloc.kind == "ExternalOutput":
                shape = tuple(alloc.tensor_shape)
                dtype = mybir.dt.np(alloc.dtype)
                out_names.append(name)
                out_avals.append(jax.core.ShapedArray(shape, dtype))
                zero_outs.append(np.zeros(shape, dtype))
        self.in_names, self.out_names = in_names, out_names
        self.out_avals, self.zero_outs = out_avals, zero_outs
        all_in = in_names + out_names + ([partition_name] if partition_name else [])

        def _body(*args):
            operands = list(args)
            if partition_name is not None:
                operands.append(partition_id_tensor())
            return tuple(_bass_exec_p.bind(
                *operands,
                out_avals=tuple(out_avals),
                in_names=tuple(all_in),
                out_names=tuple(out_names),
                lowering_input_output_aliases=(),
                sim_require_finite=True,
                sim_require_nnan=True,
                nc=nc,
            ))

        devices = jax.devices()[:N_CORES]
        self.mesh = Mesh(np.asarray(devices), ("core",))
        n_args = len(in_names) + len(out_names)
        self.fn = jax.jit(
            shard_map(_body, mesh=self.mesh,
                      in_specs=(PartitionSpec("core"),) * n_args,
                      out_specs=(PartitionSpec("core"),) * len(out_names),
                      check_rep=False),
            keep_unused=True,
        )

    def place_inputs(self, in_maps):
        from jax.sharding import NamedSharding, PartitionSpec
        sharding = NamedSharding(self.mesh, PartitionSpec("core"))
        args = []
        for name in self.in_names:
            concat = np.concatenate(
                [np.asarray(m[name]) for m in in_maps], axis=0)
            args.append(self.jax.device_put(concat, sharding))
        for z in self.zero_outs:
            concat = np.zeros((N_CORES * z.shape[0], *z.shape[1:]), z.dtype)
            args.append(self.jax.device_put(concat, sharding))
        return args

    def run(self, args):
        import time
        last = None
        for attempt in range(3):
            try:
                outs = self.fn(*args)
                self.jax.block_until_ready(outs)
                return outs
            except Exception as e:  # transient device-state errors: retry
                last = e
                time.sleep(2.0 * (attempt + 1))
        raise last

    def results(self, outs):
        res = []
        for c in range(N_CORES):
            d = {}
            for i, name in enumerate(self.out_names):
                d[name] = np.asarray(outs[i]).reshape(
                    N_CORES, *self.out_avals[i].shape)[c]
            res.append(d)
        return res


_CACHE = {}


def _get_runner(Nc, ntiles):
    key = (Nc, ntiles)
    if key not in _CACHE:
        nc = _build_program(Nc, ntiles)
        _CACHE[key] = _Runner(nc)
    return _CACHE[key]


def _make_in_maps(x, batch, W1, b1, W2):
    xT, S, Nc, ntiles, npad, seg_order = _prep_host(x, batch)
    W1 = np.asarray(W1, np.float32)
    W2 = np.asarray(W2, np.float32)
    b1 = np.asarray(b1, np.float32)
    w1t = np.ascontiguousarray(W1.T).astype(_BF16)       # [128, 256]
    w2t = np.ascontiguousarray(W2.T).reshape(2, H, H).astype(_BF16)
    b1c = b1.reshape(2, H, 1)
    in_maps = []
    for c in range(N_CORES):
        in_maps.append({
            "xt": xT[c], "w1t": w1t, "w2t": w2t, "b1c": b1c, "smat": S[c],
        })
    return in_maps, Nc, ntiles, npad, seg_order


def kernel(x, batch, W1, b1, W2, b2):
    x = np.asarray(x, np.float32)
    batch_np = np.asarray(batch)
    b1_np = np.asarray(b1, np.float32)
    b2_np = np.asarray(b2, np.float32)
    W2_np = np.asarray(W2, np.float32)

    in_maps, Nc, ntiles, npad, seg_order = _make_in_maps(x, batch_np, W1, b1_np, W2_np)
    runner = _get_runner(Nc, ntiles)
    args = runner.place_inputs(in_maps)
    outs = runner.run(args)
    res = runner.results(outs)

    yrows = np.concatenate([res[c]["y"] for c in range(N_CORES)], axis=0)
    y = np.empty((G, H), np.float32)
    y[seg_order] = yrows                       # un-permute window packing
    corr = (np.tanh(b1_np.astype(np.float64))
            @ W2_np.astype(np.float64).T).astype(np.float32)
    y = y + b2_np[None, :] - npad[:, None] * corr[None, :]
    return y.astype(np.float32)

